# revision 28
# baseline (speedup 1.0000x reference)
"""CRF loss (forward-algorithm log-partition minus gold-path score) on 8 trn2 cores.

Strategy (data-parallel over B, 32 rows per core):

  Denominator via a split scan that halves the serial-latency chain:
    Z_b = 1^T M_l .. M_1 a_0  (l = len_b - 1, M_t = diag(e_t) E^T,
    e_t = exp(emit_t - SHIFT), E = exp(transitions), a_0 = e_0).
    * forward half  (t = 1..255):   a <- e_t * (E^T a)         [true alpha_255]
    * backward half (t = 511..256): r <- E (e~_t * r) + (1-m_t)
      where e~_t is e_t with masked steps zeroed (host bakes -1000 into
      the masked emissions so exp underflows to 0) and m_t is the mask.
      Masked steps therefore compute r <- 0 + 1, the correct "inactive"
      suffix state, with no select op; the +(1-m_t) enters as a rank-1
      K=1 matmul accumulated into the same PSUM group (issued first so
      it prefetches off the critical path). r never materializes: the
      PSUM->SBUF move doubles as the next step's emission multiply.
    Both chains advance in one fused DVE multiply per step over a
    paired emission layout em_pair[c, k, {fwd,bwd}, b], so each scan
    step is two independent matmuls + ONE tensor_tensor.
    Z_b = <r_256, a_255> per row, one dot; SHIFT contributes exactly
    SHIFT*len_b per row, added back on the host.
  All scan arithmetic is bf16 on the PE/DVE path with fp32 PSUM.

  Numerator: one-hot matmuls, one per (b, t-chunk): lhsT = OH(tags),
  rhs = [emissions_chunk | OH(tags_next)] concatenated [128, 256],
  accumulated over all 128 iterations into a single PSUM tile; a single
  Frobenius product with [I | transitions] then yields emit + trans
  scores summed.

Output per core: scalar sum over its rows of (log Z~_b - log_num_b);
host adds SHIFT*sum(len)/B and divides by B.
"""

import numpy as np
import ml_dtypes

B, T, C = 256, 512, 128
NCORES = 8
BL = B // NCORES
TH = T // 2           # split point: fwd covers t<TH via alpha, bwd t>=TH
K = T - TH            # scan iterations (256)
SHIFT = float(np.log(128.0) + 0.5)  # cancels E[log sum_j exp(em_j)] per step
NCH = T // 128        # 4 numerator t-chunks

_cache = {}


def _build_program():
    import concourse.bass as bass
    import concourse.bacc as bacc
    import concourse.tile as tile
    from concourse import mybir

    f32 = mybir.dt.float32
    bf16 = mybir.dt.bfloat16
    Alu = mybir.AluOpType
    Act = mybir.ActivationFunctionType
    Axis = mybir.AxisListType

    nc = bacc.Bacc(None)

    em_pair = nc.dram_tensor("em_pair", [C, K, 2, BL], bf16, kind="ExternalInput")
    em_btc = nc.dram_tensor("em_btc", [BL, T, C], bf16, kind="ExternalInput")
    tagsm_tb = nc.dram_tensor("tagsm_tb", [T, BL], f32, kind="ExternalInput")
    tagsms_tb = nc.dram_tensor("tagsms_tb", [T, BL], f32, kind="ExternalInput")
    onem_tb = nc.dram_tensor("onem_tb", [K, BL], bf16, kind="ExternalInput")
    trans_in = nc.dram_tensor("trans", [C, C], f32, kind="ExternalInput")
    transT_in = nc.dram_tensor("transT", [C, C], f32, kind="ExternalInput")
    out_d = nc.dram_tensor("out", [1, 1], f32, kind="ExternalOutput")

    ident_in = nc.inline_tensor(np.eye(C, dtype=np.float32), name="ident")
    onesb_in = nc.inline_tensor(
        np.ones((C, 1), ml_dtypes.bfloat16), name="onesbf"
    )
    onesrow_in = nc.inline_tensor(
        np.ones((1, C), ml_dtypes.bfloat16), name="onesrow"
    )
    onesf_in = nc.inline_tensor(np.ones((C, 1), np.float32), name="onesf")
    iota4_in = nc.inline_tensor(
        np.broadcast_to(
            np.arange(C, dtype=np.float32), (C, NCH, C)
        ).copy(),
        name="iota4",
    )

    with tile.TileContext(nc) as tc:
        with (
            tc.tile_pool(name="consts", bufs=1) as consts,
            tc.tile_pool(name="bigbuf", bufs=1) as bigbuf,
            tc.tile_pool(name="sp", bufs=3) as sppool,
            tc.tile_pool(name="ups", bufs=2, space="PSUM") as ups,
            tc.tile_pool(name="accps", bufs=1, space="PSUM") as accps,
            tc.tile_pool(name="rsps", bufs=2, space="PSUM") as rsps,
            tc.tile_pool(name="oh", bufs=3) as ohpool,
            tc.tile_pool(name="combo", bufs=3) as combopool,
        ):
            # ---------- scan-critical constants first ----------
            neg_shift = consts.tile([C, 1], f32)
            nc.vector.memset(neg_shift[:], -SHIFT)
            # ---------- emissions: exp(em - SHIFT), bf16, paired layout ----
            # exp_em[:, k, 0, :] = e_(k+1)   (fwd multiplier at iter k)
            # exp_em[:, k, 1, :] = e~_(510-k) (bwd multiplier at iter k)
            # exp_em[:, K-1, 0, :] = e_0 ; exp_em[:, K-1, 1, :] = e~_511
            exp_em = bigbuf.tile([C, K, 2, BL], bf16)
            # init slice first (chain heads), then geometrically growing
            # chunks in k order so the scan can start almost immediately
            chunks = [
                (K - 1, K), (0, 4), (4, 12), (12, 28), (28, 56),
                (56, 88), (88, 120), (120, 152), (152, 184),
                (184, 216), (216, K - 1),
            ]

            def emit_chunk(lo, hi):
                nc.sync.dma_start(
                    out=exp_em[:, lo:hi, :, :], in_=em_pair[:, lo:hi, :, :]
                )
                nc.scalar.activation(
                    out=exp_em[:, lo:hi, :, :],
                    in_=exp_em[:, lo:hi, :, :],
                    func=Act.Exp, bias=neg_shift[:], scale=1.0,
                )

            for lo, hi in chunks[:2]:
                emit_chunk(lo, hi)

            # idtr = [I | transitions]  (fp32) for the final Frobenius
            idtr = consts.tile([C, 2 * C], f32)
            nc.sync.dma_start(out=idtr[:, C : 2 * C], in_=trans_in[:])
            E_bf = consts.tile([C, C], bf16)
            nc.scalar.activation(out=E_bf[:], in_=idtr[:, C : 2 * C], func=Act.Exp)
            transT_sb = consts.tile([C, C], f32)
            nc.sync.dma_start(out=transT_sb[:], in_=transT_in[:])
            ET_bf = consts.tile([C, C], bf16)
            nc.scalar.activation(out=ET_bf[:], in_=transT_sb[:], func=Act.Exp)
            ones_row = consts.tile([1, C], bf16)
            nc.sync.dma_start(out=ones_row[:], in_=onesrow_in[:])
            # 1 - mask for t in [TH, T): index j holds t = T-1-j (K=1 rhs)
            onem_sb = consts.tile([1, K, BL], bf16)
            nc.sync.dma_start(out=onem_sb[:], in_=onem_tb[:])

            for lo, hi in chunks[2:5]:
                emit_chunk(lo, hi)

            # ---------- remaining constants ----------
            nc.sync.dma_start(out=idtr[:, 0:C], in_=ident_in[:])
            ones_bf = consts.tile([C, 1], bf16)
            nc.sync.dma_start(out=ones_bf[:], in_=onesb_in[:])
            ones_f = consts.tile([C, 1], f32)
            nc.sync.dma_start(out=ones_f[:], in_=onesf_in[:])
            iota4_sb = consts.tile([C, NCH, C], f32)
            nc.sync.dma_start(out=iota4_sb[:], in_=iota4_in[:])

            tags_m = consts.tile([128, NCH, BL], f32)
            nc.sync.dma_start(
                out=tags_m[:],
                in_=tagsm_tb[:].rearrange("(h l) b -> l h b", l=128),
            )
            tags_ms = consts.tile([128, NCH, BL], f32)
            nc.sync.dma_start(
                out=tags_ms[:],
                in_=tagsms_tb[:].rearrange("(h l) b -> l h b", l=128),
            )

            for lo, hi in chunks[5:]:
                emit_chunk(lo, hi)

            # ---------- the fused split scan ----------
            sp_prev = None
            up_last = None
            for k in range(K):
                up = ups.tile([C, 2, BL], f32, tag="up")
                # bwd: p = E (e~*r) + (1-m);  rank-1 first (prefetches)
                nc.tensor.matmul(
                    up[:, 1, :], lhsT=ones_row[:], rhs=onem_sb[:, k, :],
                    start=True, stop=False, skip_group_check=True,
                )
                rhs_b = (
                    exp_em[:, K - 1, 1, :] if k == 0 else sp_prev[:, 1, :]
                )
                nc.tensor.matmul(
                    up[:, 1, :], lhsT=ET_bf[:], rhs=rhs_b,
                    start=False, stop=True, skip_group_check=True,
                )
                if k < K - 1:
                    # fwd: u = E^T a
                    rhs_f = (
                        exp_em[:, K - 1, 0, :] if k == 0 else sp_prev[:, 0, :]
                    )
                    nc.tensor.matmul(
                        up[:, 0, :], lhsT=E_bf[:], rhs=rhs_f,
                        start=True, stop=True, skip_group_check=True,
                    )
                    # one fused multiply advances both chains
                    sp = sppool.tile([C, 2, BL], bf16, tag="sp")
                    nc.vector.tensor_tensor(
                        out=sp[:], in0=up[:], in1=exp_em[:, k, :, :],
                        op=Alu.mult,
                    )
                    sp_prev = sp
                else:
                    up_last = up

            # ---------- denominator: per-row dot + log ----------
            # Z_b = <p(TH), a(TH-1)>  (p = r exactly, rank-1 term included)
            d = consts.tile([C, BL], bf16)
            nc.vector.tensor_tensor(
                out=d[:], in0=up_last[:, 1, :], in1=sp_prev[:, 0, :],
                op=Alu.mult,
            )
            dot_ps = rsps.tile([1, BL], f32, tag="rs")
            nc.tensor.matmul(
                dot_ps[:], lhsT=ones_bf[:, :1], rhs=d[:], start=True, stop=True
            )
            logd = consts.tile([1, BL], f32)
            nc.scalar.activation(out=logd[:], in_=dot_ps[:], func=Act.Ln)
            den_s = consts.tile([1, 1], f32)
            nc.vector.tensor_reduce(
                out=den_s[:], in_=logd[:], axis=Axis.X, op=Alu.add
            )

            # ---------- numerator: one-hot matmuls ----------
            acc_ps = accps.tile([C, 2 * C], f32)
            for b in range(BL):
                # small-quantum is_eq builds (FD=128) slot into the DVE gaps
                # between scan multiplies without delaying them
                oh = ohpool.tile([128, NCH, C], bf16, tag="oh")
                combo = combopool.tile([128, NCH, 2 * C], bf16, tag="combo")
                # combo em-half DMA triggered from the idle GpSimd engine so
                # the Sync queue stays clear for scan-critical transfers
                nc.gpsimd.dma_start(
                    out=combo[:, :, 0:C],
                    in_=em_btc[b].rearrange("(h l) c -> l h c", l=128),
                )
                for ch in range(NCH):
                    nc.vector.tensor_tensor(
                        out=oh[:, ch, :], in0=iota4_sb[:, ch, :],
                        in1=tags_m[:, ch, b : b + 1].to_broadcast([128, C]),
                        op=Alu.is_equal,
                    )
                    nc.vector.tensor_tensor(
                        out=combo[:, ch, C : 2 * C], in0=iota4_sb[:, ch, :],
                        in1=tags_ms[:, ch, b : b + 1].to_broadcast([128, C]),
                        op=Alu.is_equal,
                    )
                for ch in range(NCH):
                    i = b * NCH + ch
                    nc.tensor.matmul(
                        acc_ps[:], lhsT=oh[:, ch, :], rhs=combo[:, ch, :],
                        start=(i == 0), stop=(i == BL * NCH - 1),
                        skip_group_check=True,
                    )

            # ---------- numerator frobenius ([I | trans] in one shot) ----------
            frob = consts.tile([C, 2 * C], f32)
            nc.vector.tensor_tensor(
                out=frob[:], in0=acc_ps[:], in1=idtr[:], op=Alu.mult
            )
            num_acc = consts.tile([128, 1], f32)
            nc.vector.tensor_reduce(
                out=num_acc[:], in_=frob[:], axis=Axis.X, op=Alu.add
            )
            num_ps = rsps.tile([1, 1], f32, tag="rs")
            nc.tensor.matmul(
                num_ps[:], lhsT=ones_f[:, :1], rhs=num_acc[:],
                start=True, stop=True,
            )

            # ---------- final scalar ----------
            res_sb = consts.tile([1, 1], f32)
            nc.vector.tensor_tensor(
                out=res_sb[:], in0=den_s[:], in1=num_ps[:], op=Alu.subtract
            )
            nc.sync.dma_start(out=out_d[:], in_=res_sb[:])

    nc.compile()
    return nc


def _prep_inputs(emissions, tags, mask, transitions):
    em = np.asarray(emissions)
    tg = np.asarray(tags).astype(np.int32)
    mk = np.asarray(mask).astype(bool)
    tr = np.ascontiguousarray(np.asarray(transitions), dtype=np.float32)
    trT = np.ascontiguousarray(tr.T)

    # paired time index: slot0 -> t = k+1 (k<K-1), t=0 at k=K-1
    #                    slot1 -> t = 510-k (k<K-1), t=511 at k=K-1
    t_fwd = np.concatenate([np.arange(1, TH), [0]])
    t_bwd = np.concatenate([np.arange(T - 2, TH - 1, -1), [T - 1]])

    in_maps = []
    for core in range(NCORES):
        b0, b1 = core * BL, (core + 1) * BL
        em_c = np.asarray(em[b0:b1], dtype=np.float32)
        mk_c3 = mk[b0:b1][:, :, None]                 # [BL, T, 1]
        em_masked = np.where(mk_c3, em_c, -1000.0).astype(np.float32)
        em_ctb = em_masked.transpose(2, 1, 0)         # [C, T, BL]
        em_pair = np.stack(
            [em_ctb[:, t_fwd, :], em_ctb[:, t_bwd, :]], axis=2
        )                                             # [C, K, 2, BL]
        tg_c = tg[b0:b1].T                            # [T, BL] int32
        mk_c = mk[b0:b1].T.astype(np.float32)         # [T, BL]
        pad_f = np.zeros((1, BL), np.float32)

        # masked tags (+1000 where mask off) for the one-hot builds
        tags_m = (tg_c + 1000.0 * (1.0 - mk_c)).astype(np.float32)
        tg_next = np.vstack([tg_c[1:], np.zeros((1, BL), np.int32)])
        mk_next = np.vstack([mk_c[1:], pad_f])
        tags_ms = (tg_next + 1000.0 * (1.0 - mk_next)).astype(np.float32)

        # onem_sb[0, j, b] = 1 - mask[t = T-1-j]  (rank-1 rhs at iter k=j)
        onem = (1.0 - mk_c[T - 1 : TH - 1 : -1]).astype(ml_dtypes.bfloat16)

        in_maps.append({
            "em_pair": np.ascontiguousarray(em_pair).astype(ml_dtypes.bfloat16),
            "em_btc": np.ascontiguousarray(em_c).astype(ml_dtypes.bfloat16),
            "tagsm_tb": np.ascontiguousarray(tags_m),
            "tagsms_tb": np.ascontiguousarray(tags_ms),
            "onem_tb": np.ascontiguousarray(onem),
            "trans": tr,
            "transT": trT,
        })
    return in_maps


def kernel(emissions, tags, mask, transitions, _want_results=False, **_run_kw):
    from concourse.bass_utils import run_bass_kernel_spmd

    if "nc" not in _cache:
        _cache["nc"] = _build_program()
    nc = _cache["nc"]

    in_maps = _prep_inputs(emissions, tags, mask, transitions)
    res = run_bass_kernel_spmd(nc, in_maps, core_ids=list(range(NCORES)), **_run_kw)
    total = sum(float(r["out"][0, 0]) for r in res.results)
    lengths_total = int(np.asarray(mask).astype(np.int64).sum())
    out = np.float32((total + SHIFT * lengths_total) / B)
    if _want_results:
        return out, res
    return out


# revision 29
# speedup vs baseline: 1.0076x; 1.0076x over previous
"""CRF loss (forward-algorithm log-partition minus gold-path score) on 8 trn2 cores.

Strategy (data-parallel over B, 32 rows per core):

  Denominator via a split scan that halves the serial-latency chain:
    Z_b = 1^T M_l .. M_1 a_0  (l = len_b - 1, M_t = diag(e_t) E^T,
    e_t = exp(emit_t - SHIFT), E = exp(transitions), a_0 = e_0).
    * forward half  (t = 1..255):   a <- e_t * (E^T a)         [true alpha_255]
    * backward half (t = 511..256): r <- E (e~_t * r) + (1-m_t)
      where e~_t is e_t with masked steps zeroed (host bakes -1000 into
      the masked emissions so exp underflows to 0) and m_t is the mask.
      Masked steps therefore compute r <- 0 + 1, the correct "inactive"
      suffix state, with no select op; the +(1-m_t) enters as a rank-1
      K=1 matmul accumulated into the same PSUM group (issued first so
      it prefetches off the critical path). r never materializes: the
      PSUM->SBUF move doubles as the next step's emission multiply.
    Both chains advance in one fused DVE multiply per step over a
    paired emission layout em_pair[c, k, {fwd,bwd}, b], so each scan
    step is two independent matmuls + ONE tensor_tensor.
    Z_b = <r_256, a_255> per row, one dot; SHIFT contributes exactly
    SHIFT*len_b per row, added back on the host.
  All scan arithmetic is bf16 on the PE/DVE path with fp32 PSUM.

  Numerator: one-hot matmuls, one per (b, t-chunk): lhsT = OH(tags),
  rhs = [emissions_chunk | OH(tags_next)] concatenated [128, 256],
  accumulated over all 128 iterations into a single PSUM tile; a single
  Frobenius product with [I | transitions] then yields emit + trans
  scores summed.

Output per core: scalar sum over its rows of (log Z~_b - log_num_b);
host adds SHIFT*sum(len)/B and divides by B.
"""

import numpy as np
import ml_dtypes

B, T, C = 256, 512, 128
NCORES = 8
BL = B // NCORES
TH = T // 2           # split point: fwd covers t<TH via alpha, bwd t>=TH
K = T - TH            # scan iterations (256)
SHIFT = float(np.log(128.0) + 0.5)  # cancels E[log sum_j exp(em_j)] per step
NCH = T // 128        # 4 numerator t-chunks

_cache = {}


def _build_program():
    import concourse.bass as bass
    import concourse.bacc as bacc
    import concourse.tile as tile
    from concourse import mybir

    f32 = mybir.dt.float32
    bf16 = mybir.dt.bfloat16
    Alu = mybir.AluOpType
    Act = mybir.ActivationFunctionType
    Axis = mybir.AxisListType

    nc = bacc.Bacc(None)

    em_pair = nc.dram_tensor("em_pair", [C, K, 2, BL], bf16, kind="ExternalInput")
    em_btc = nc.dram_tensor("em_btc", [BL, T, C], bf16, kind="ExternalInput")
    tagsm_tb = nc.dram_tensor("tagsm_tb", [T, BL], f32, kind="ExternalInput")
    tagsms_tb = nc.dram_tensor("tagsms_tb", [T, BL], f32, kind="ExternalInput")
    onem_tb = nc.dram_tensor("onem_tb", [K, BL], bf16, kind="ExternalInput")
    trans2_in = nc.dram_tensor("trans2", [C, 2 * C], f32, kind="ExternalInput")
    out_d = nc.dram_tensor("out", [1, 1], f32, kind="ExternalOutput")

    ident_in = nc.inline_tensor(np.eye(C, dtype=np.float32), name="ident")
    onesb_in = nc.inline_tensor(
        np.ones((C, 1), ml_dtypes.bfloat16), name="onesbf"
    )
    onesrow_in = nc.inline_tensor(
        np.ones((1, C), ml_dtypes.bfloat16), name="onesrow"
    )
    onesf_in = nc.inline_tensor(np.ones((C, 1), np.float32), name="onesf")
    iota4_in = nc.inline_tensor(
        np.broadcast_to(
            np.arange(C, dtype=np.float32), (C, NCH, C)
        ).copy(),
        name="iota4",
    )

    with tile.TileContext(nc) as tc:
        with (
            tc.tile_pool(name="consts", bufs=1) as consts,
            tc.tile_pool(name="bigbuf", bufs=1) as bigbuf,
            tc.tile_pool(name="sp", bufs=3) as sppool,
            tc.tile_pool(name="ups", bufs=2, space="PSUM") as ups,
            tc.tile_pool(name="accps", bufs=1, space="PSUM") as accps,
            tc.tile_pool(name="rsps", bufs=2, space="PSUM") as rsps,
            tc.tile_pool(name="oh", bufs=3) as ohpool,
            tc.tile_pool(name="combo", bufs=3) as combopool,
        ):
            # ---------- scan-critical constants first ----------
            neg_shift = consts.tile([C, 1], f32)
            nc.vector.memset(neg_shift[:], -SHIFT)
            # ---------- emissions: exp(em - SHIFT), bf16, paired layout ----
            # exp_em[:, k, 0, :] = e_(k+1)   (fwd multiplier at iter k)
            # exp_em[:, k, 1, :] = e~_(510-k) (bwd multiplier at iter k)
            # exp_em[:, K-1, 0, :] = e_0 ; exp_em[:, K-1, 1, :] = e~_511
            exp_em = bigbuf.tile([C, K, 2, BL], bf16)
            # init slice first (chain heads), then geometrically growing
            # chunks in k order so the scan can start almost immediately
            chunks = [
                (K - 1, K), (0, 4), (4, 12), (12, 28), (28, 56),
                (56, 88), (88, 120), (120, 152), (152, 184),
                (184, 216), (216, K - 1),
            ]

            def emit_chunk(lo, hi):
                nc.sync.dma_start(
                    out=exp_em[:, lo:hi, :, :], in_=em_pair[:, lo:hi, :, :]
                )
                nc.scalar.activation(
                    out=exp_em[:, lo:hi, :, :],
                    in_=exp_em[:, lo:hi, :, :],
                    func=Act.Exp, bias=neg_shift[:], scale=1.0,
                )

            for lo, hi in chunks[:2]:
                emit_chunk(lo, hi)

            # [E | E^T] in one DMA + one exp (E_bf/ET_bf are slices)
            trans2_sb = consts.tile([C, 2 * C], f32)
            nc.sync.dma_start(out=trans2_sb[:], in_=trans2_in[:])
            EET_bf = consts.tile([C, 2 * C], bf16)
            nc.scalar.activation(out=EET_bf[:], in_=trans2_sb[:], func=Act.Exp)
            E_bf = EET_bf[:, 0:C]
            ET_bf = EET_bf[:, C : 2 * C]
            ones_row = consts.tile([1, C], bf16)
            nc.sync.dma_start(out=ones_row[:], in_=onesrow_in[:])
            # 1 - mask for t in [TH, T): index j holds t = T-1-j (K=1 rhs)
            onem_sb = consts.tile([1, K, BL], bf16)
            nc.sync.dma_start(out=onem_sb[:], in_=onem_tb[:])

            for lo, hi in chunks[2:5]:
                emit_chunk(lo, hi)

            # ---------- remaining constants ----------
            # idtr = [I | transitions] (fp32), only needed by the final
            # Frobenius, so its loads live off the critical path
            idtr = consts.tile([C, 2 * C], f32)
            nc.sync.dma_start(out=idtr[:, 0:C], in_=ident_in[:])
            nc.sync.dma_start(out=idtr[:, C : 2 * C], in_=trans2_in[:, 0:C])
            ones_bf = consts.tile([C, 1], bf16)
            nc.sync.dma_start(out=ones_bf[:], in_=onesb_in[:])
            ones_f = consts.tile([C, 1], f32)
            nc.sync.dma_start(out=ones_f[:], in_=onesf_in[:])
            iota4_sb = consts.tile([C, NCH, C], f32)
            nc.sync.dma_start(out=iota4_sb[:], in_=iota4_in[:])

            tags_m = consts.tile([128, NCH, BL], f32)
            nc.sync.dma_start(
                out=tags_m[:],
                in_=tagsm_tb[:].rearrange("(h l) b -> l h b", l=128),
            )
            tags_ms = consts.tile([128, NCH, BL], f32)
            nc.sync.dma_start(
                out=tags_ms[:],
                in_=tagsms_tb[:].rearrange("(h l) b -> l h b", l=128),
            )

            for lo, hi in chunks[5:]:
                emit_chunk(lo, hi)

            # ---------- the fused split scan ----------
            sp_prev = None
            up_last = None
            for k in range(K):
                up = ups.tile([C, 2, BL], f32, tag="up")
                # bwd: p = E (e~*r) + (1-m);  rank-1 first (prefetches)
                nc.tensor.matmul(
                    up[:, 1, :], lhsT=ones_row[:], rhs=onem_sb[:, k, :],
                    start=True, stop=False, skip_group_check=True,
                )
                rhs_b = (
                    exp_em[:, K - 1, 1, :] if k == 0 else sp_prev[:, 1, :]
                )
                nc.tensor.matmul(
                    up[:, 1, :], lhsT=ET_bf, rhs=rhs_b,
                    start=False, stop=True, skip_group_check=True,
                )
                if k < K - 1:
                    # fwd: u = E^T a
                    rhs_f = (
                        exp_em[:, K - 1, 0, :] if k == 0 else sp_prev[:, 0, :]
                    )
                    nc.tensor.matmul(
                        up[:, 0, :], lhsT=E_bf, rhs=rhs_f,
                        start=True, stop=True, skip_group_check=True,
                    )
                    # one fused multiply advances both chains
                    sp = sppool.tile([C, 2, BL], bf16, tag="sp")
                    nc.vector.tensor_tensor(
                        out=sp[:], in0=up[:], in1=exp_em[:, k, :, :],
                        op=Alu.mult,
                    )
                    sp_prev = sp
                else:
                    up_last = up

            # ---------- denominator: per-row dot + log ----------
            # Z_b = <p(TH), a(TH-1)>  (p = r exactly, rank-1 term included)
            d = consts.tile([C, BL], bf16)
            nc.vector.tensor_tensor(
                out=d[:], in0=up_last[:, 1, :], in1=sp_prev[:, 0, :],
                op=Alu.mult,
            )
            dot_ps = rsps.tile([1, BL], f32, tag="rs")
            nc.tensor.matmul(
                dot_ps[:], lhsT=ones_bf[:, :1], rhs=d[:], start=True, stop=True
            )
            logd = consts.tile([1, BL], f32)
            nc.scalar.activation(out=logd[:], in_=dot_ps[:], func=Act.Ln)
            den_s = consts.tile([1, 1], f32)
            nc.vector.tensor_reduce(
                out=den_s[:], in_=logd[:], axis=Axis.X, op=Alu.add
            )

            # ---------- numerator: one-hot matmuls ----------
            acc_ps = accps.tile([C, 2 * C], f32)
            for b in range(BL):
                # small-quantum is_eq builds (FD=128) slot into the DVE gaps
                # between scan multiplies without delaying them
                oh = ohpool.tile([128, NCH, C], bf16, tag="oh")
                combo = combopool.tile([128, NCH, 2 * C], bf16, tag="combo")
                # combo em-half DMA triggered from the idle GpSimd engine so
                # the Sync queue stays clear for scan-critical transfers
                nc.gpsimd.dma_start(
                    out=combo[:, :, 0:C],
                    in_=em_btc[b].rearrange("(h l) c -> l h c", l=128),
                )
                for ch in range(NCH):
                    nc.vector.tensor_tensor(
                        out=oh[:, ch, :], in0=iota4_sb[:, ch, :],
                        in1=tags_m[:, ch, b : b + 1].to_broadcast([128, C]),
                        op=Alu.is_equal,
                    )
                    nc.vector.tensor_tensor(
                        out=combo[:, ch, C : 2 * C], in0=iota4_sb[:, ch, :],
                        in1=tags_ms[:, ch, b : b + 1].to_broadcast([128, C]),
                        op=Alu.is_equal,
                    )
                for ch in range(NCH):
                    i = b * NCH + ch
                    nc.tensor.matmul(
                        acc_ps[:], lhsT=oh[:, ch, :], rhs=combo[:, ch, :],
                        start=(i == 0), stop=(i == BL * NCH - 1),
                        skip_group_check=True,
                    )

            # ---------- numerator frobenius ([I | trans] in one shot) ----------
            frob = consts.tile([C, 2 * C], f32)
            nc.vector.tensor_tensor(
                out=frob[:], in0=acc_ps[:], in1=idtr[:], op=Alu.mult
            )
            num_acc = consts.tile([128, 1], f32)
            nc.vector.tensor_reduce(
                out=num_acc[:], in_=frob[:], axis=Axis.X, op=Alu.add
            )
            num_ps = rsps.tile([1, 1], f32, tag="rs")
            nc.tensor.matmul(
                num_ps[:], lhsT=ones_f[:, :1], rhs=num_acc[:],
                start=True, stop=True,
            )

            # ---------- final scalar ----------
            res_sb = consts.tile([1, 1], f32)
            nc.vector.tensor_tensor(
                out=res_sb[:], in0=den_s[:], in1=num_ps[:], op=Alu.subtract
            )
            nc.sync.dma_start(out=out_d[:], in_=res_sb[:])

    nc.compile()
    return nc


def _prep_inputs(emissions, tags, mask, transitions):
    em = np.asarray(emissions)
    tg = np.asarray(tags).astype(np.int32)
    mk = np.asarray(mask).astype(bool)
    tr = np.ascontiguousarray(np.asarray(transitions), dtype=np.float32)
    tr2 = np.ascontiguousarray(np.hstack([tr, tr.T]))

    # paired time index: slot0 -> t = k+1 (k<K-1), t=0 at k=K-1
    #                    slot1 -> t = 510-k (k<K-1), t=511 at k=K-1
    t_fwd = np.concatenate([np.arange(1, TH), [0]])
    t_bwd = np.concatenate([np.arange(T - 2, TH - 1, -1), [T - 1]])

    in_maps = []
    for core in range(NCORES):
        b0, b1 = core * BL, (core + 1) * BL
        em_c = np.asarray(em[b0:b1], dtype=np.float32)
        mk_c3 = mk[b0:b1][:, :, None]                 # [BL, T, 1]
        em_masked = np.where(mk_c3, em_c, -1000.0).astype(np.float32)
        em_ctb = em_masked.transpose(2, 1, 0)         # [C, T, BL]
        em_pair = np.stack(
            [em_ctb[:, t_fwd, :], em_ctb[:, t_bwd, :]], axis=2
        )                                             # [C, K, 2, BL]
        tg_c = tg[b0:b1].T                            # [T, BL] int32
        mk_c = mk[b0:b1].T.astype(np.float32)         # [T, BL]
        pad_f = np.zeros((1, BL), np.float32)

        # masked tags (+1000 where mask off) for the one-hot builds
        tags_m = (tg_c + 1000.0 * (1.0 - mk_c)).astype(np.float32)
        tg_next = np.vstack([tg_c[1:], np.zeros((1, BL), np.int32)])
        mk_next = np.vstack([mk_c[1:], pad_f])
        tags_ms = (tg_next + 1000.0 * (1.0 - mk_next)).astype(np.float32)

        # onem_sb[0, j, b] = 1 - mask[t = T-1-j]  (rank-1 rhs at iter k=j)
        onem = (1.0 - mk_c[T - 1 : TH - 1 : -1]).astype(ml_dtypes.bfloat16)

        in_maps.append({
            "em_pair": np.ascontiguousarray(em_pair).astype(ml_dtypes.bfloat16),
            "em_btc": np.ascontiguousarray(em_c).astype(ml_dtypes.bfloat16),
            "tagsm_tb": np.ascontiguousarray(tags_m),
            "tagsms_tb": np.ascontiguousarray(tags_ms),
            "onem_tb": np.ascontiguousarray(onem),
            "trans2": tr2,
        })
    return in_maps


def kernel(emissions, tags, mask, transitions, _want_results=False, **_run_kw):
    from concourse.bass_utils import run_bass_kernel_spmd

    if "nc" not in _cache:
        _cache["nc"] = _build_program()
    nc = _cache["nc"]

    in_maps = _prep_inputs(emissions, tags, mask, transitions)
    res = run_bass_kernel_spmd(nc, in_maps, core_ids=list(range(NCORES)), **_run_kw)
    total = sum(float(r["out"][0, 0]) for r in res.results)
    lengths_total = int(np.asarray(mask).astype(np.int64).sum())
    out = np.float32((total + SHIFT * lengths_total) / B)
    if _want_results:
        return out, res
    return out
